# revision 31
# baseline (speedup 1.0000x reference)
"""LSTM decoder (B=16384, H=256, I=O=2, T=60) on 8 TRN2 NeuronCores.

Strategy: pure data parallel over the batch dim (2048 rows/core). All state
kept on-chip in transposed [H, B_local] layout so the recurrent matmul needs
no transposes. The output-feedback path (y_t -> x_{t+1}) is folded into the
recurrent weights: for t>=1,
    gates_t = h_t @ (W_hh + W_ih @ fc_W).T + (b_ih + b_hh + W_ih @ fc_b)
so each step is one [2048,256]@[256,1024] matmul (+ tiny fc head for y_t).
Gate rows are reordered to [i, f, o, g]; PSUM is split 4+2+2 banks
(i,f | o | g + y-head) so TensorE refills one pool while ScalarE drains
another, and the tanh(c)/h/y tail is software-pipelined 1-2 chunks behind
the gate matmuls. ScalarE (5 transcendentals per hidden element) is the
bottleneck engine at ~96% busy; measured ~1.21 ms for the full decode.
"""

import os
import sys
import types
import contextlib
import ctypes

sys.path.insert(0, "/opt/trn_rl_repo")

import numpy as np
import ml_dtypes

BF16_NP = ml_dtypes.bfloat16

import concourse.bass as bass
import concourse.tile as tile
from concourse import mybir
from concourse.bass import ts
from concourse.vector_clock import ScopedClock
import bass_rust

F32 = mybir.dt.float32
BF16 = mybir.dt.bfloat16
AF = mybir.ActivationFunctionType

B, H, I_DIM, O_DIM = 16384, 256, 2, 2
NCORES = 8
BL = B // NCORES          # 2048 batch rows per core
NC = 1024                 # batch chunk (one PSUM bank of bf16 gate outputs)
NCH = BL // NC            # 2 chunks per step
KT = H // 128             # 2 contraction tiles
MT = 4 * H // 128         # 8 gate row tiles

_MAX_WAITS = 1  # walrus in this container rejects >1 sem wait per instruction

TileContextFixed = tile.TileContext


def _split_multi_waits(nc, max_waits=_MAX_WAITS):
    """The pinned walrus rejects instructions carrying more than one
    semaphore wait. Split extras onto same-engine NoOps placed just before
    the instruction (same stream order => same semantics)."""
    ctr = 0
    for fn in nc.m.functions:
        for bb in fn.blocks:
            out = []
            changed = False
            for inst in bb.instructions:
                si = getattr(inst, "sync_info", None)
                waits = list(si.on_wait) if si is not None and si.on_wait else []
                if (
                    len(waits) > max_waits
                    and inst.engine != mybir.EngineType.Unassigned
                ):
                    keep = waits[-max_waits:]
                    extra = waits[:-max_waits]
                    for j in range(0, len(extra), max_waits):
                        ctr += 1
                        nop = mybir.InstNoOp(
                            name=f"waitsplit_{ctr}", engine=inst.engine
                        )
                        nop.sync_info = bass_rust.SyncInfo(
                            on_wait=extra[j : j + max_waits], on_update=[]
                        )
                        try:
                            nc.register_instruction(nop)
                        except Exception:
                            pass
                        out.append(nop)
                    si.on_wait = keep
                    inst.sync_info = si
                    changed = True
                out.append(inst)
            if changed:
                bb.instructions = out


def _build(T: int, use_bias: bool):
    nc = bass.Bass()

    wg_d = nc.declare_dram_parameter("wg", [KT, 128, 4 * H], BF16, isOutput=False)
    wy_d = nc.declare_dram_parameter("wy", [KT, 128, O_DIM], BF16, isOutput=False)
    wx_d = nc.declare_dram_parameter("wx", [I_DIM, 4 * H], BF16, isOutput=False)
    h0_d = nc.declare_dram_parameter("h0", [KT, 128, BL], BF16, isOutput=False)
    c0_d = nc.declare_dram_parameter("c0", [KT, 128, BL], F32, isOutput=False)
    x0_d = nc.declare_dram_parameter("x0", [I_DIM, BL], BF16, isOutput=False)
    if use_bias:
        bg_d = nc.declare_dram_parameter("bg", [1, 4 * H], F32, isOutput=False)
        bg0_d = nc.declare_dram_parameter("bg0", [1, 4 * H], F32, isOutput=False)
    ys_d = nc.declare_dram_parameter("ys", [T, O_DIM, BL], BF16, isOutput=True)

    with contextlib.ExitStack() as ctx:
        tc = ctx.enter_context(TileContextFixed(nc))
        consts = ctx.enter_context(tc.tile_pool(name="consts", bufs=1))
        state = ctx.enter_context(tc.tile_pool(name="state", bufs=1))
        # PSUM: 4 banks (i,f) + 2 banks (o) + 2 banks (g, shared with y head)
        sigpA = ctx.enter_context(tc.tile_pool(name="sigpA", bufs=1, space="PSUM"))
        sigpB = ctx.enter_context(tc.tile_pool(name="sigpB", bufs=1, space="PSUM"))
        tgp = ctx.enter_context(tc.tile_pool(name="tgp", bufs=1, space="PSUM"))
        sigs = ctx.enter_context(tc.tile_pool(name="sigs", bufs=4))
        tmps = ctx.enter_context(tc.tile_pool(name="tmps", bufs=3))
        youts = ctx.enter_context(tc.tile_pool(name="youts", bufs=2))

        # PE warmup: a few junk matmuls with no DMA dependency bridge the gap
        # until the input DMAs land; the real step-0 matmuls then keep the HAM
        # activity window hot so the clock reaches 2.4 GHz early.
        warm_sb = consts.tile([128, NC], BF16)
        nc.vector.memset(warm_sb[:], 0.0)
        warm_ps = tgp.tile([128, NC], BF16, tag="g_ps")
        for _ in range(4):
            nc.tensor.matmul(
                warm_ps[:], warm_sb[:, 0:128], warm_sb[:], start=True, stop=True
            )

        wg_sb = consts.tile([128, KT, 4 * H], BF16)
        wy_sb = consts.tile([128, KT, O_DIM], BF16)
        wx_sb = consts.tile([I_DIM, 4 * H], BF16)
        x0_sb = consts.tile([I_DIM, BL], BF16)
        for k in range(KT):
            nc.sync.dma_start(out=wg_sb[:, k, :], in_=wg_d[k])
        nc.sync.dma_start(out=wx_sb[:], in_=wx_d[:])
        nc.sync.dma_start(out=x0_sb[:], in_=x0_d[:])
        for k in range(KT):
            nc.sync.dma_start(out=wy_sb[:, k, :], in_=wy_d[k])
        if use_bias:
            bg_sb = consts.tile([1, 4 * H], F32)
            bg0_sb = consts.tile([1, 4 * H], F32)
            ones_sb = consts.tile([1, NC], F32)
            nc.sync.dma_start(out=bg_sb[:], in_=bg_d[:])
            nc.sync.dma_start(out=bg0_sb[:], in_=bg0_d[:])
            nc.vector.memset(ones_sb[:], 1.0)

        hA = state.tile([128, KT, BL], BF16, tag="hA")
        hB = state.tile([128, KT, BL], BF16, tag="hB")
        cS = state.tile([128, KT, BL], F32, tag="cS")
        # chunked, h/c-interleaved loads so step-0 compute (and its c-path)
        # starts as soon as the first slices land
        for n in range(NCH):
            for k in range(KT):
                nc.sync.dma_start(
                    out=hA[:, k, ts(n, NC)], in_=h0_d[k, :, ts(n, NC)]
                )
            for k in range(KT):
                nc.sync.dma_start(
                    out=cS[:, k, ts(n, NC)], in_=c0_d[k, :, ts(n, NC)]
                )

        # Software pipeline: tanh(c)/h of chunks (t, 2j..2j+1) are emitted as a
        # pair after both adds, and the y head trails one further chunk, so
        # ScalarE/VectorE never stall the gate pipeline.
        pending_tails = []  # [(t, n, o_sb, h_out, y_sb), ...]
        pending_ys = []     # fifo of (h_buf, t, n, y_sb)

        def emit_pending_y():
            if not pending_ys:
                return
            h_src, t_p, n_p, y_sb_p = pending_ys.pop(0)
            y_ps = tgp.tile([O_DIM, NC], BF16, tag="g_ps")
            for k in range(KT):
                nc.tensor.matmul(
                    y_ps[:],
                    wy_sb[:, k, :],
                    h_src[:, k, ts(n_p, NC)],
                    start=(k == 0),
                    stop=(k == KT - 1),
                )
            nc.vector.tensor_copy(y_sb_p[:, ts(n_p, NC)], y_ps[:])
            if n_p == NCH - 1:
                nc.sync.dma_start(out=ys_d[t_p], in_=y_sb_p[:])

        def emit_tail_one():
            # tanh(c') + h for the oldest pending chunk (runs one chunk late)
            t_p, n_p, o_sb_p, h_out_p, y_sb_p = pending_tails.pop(0)
            tc_t = tmps.tile([128, KT, NC], BF16, tag="tc_t")
            nc.scalar.activation(tc_t[:], cS[:, :, ts(n_p, NC)], AF.Tanh)
            nc.vector.tensor_mul(
                h_out_p[:, :, ts(n_p, NC)], o_sb_p[:], tc_t[:]
            )
            pending_ys.append((h_out_p, t_p, n_p, y_sb_p))

        y_sb = None
        for t in range(T):
            h_in = hA if t % 2 == 0 else hB
            h_out = hB if t % 2 == 0 else hA
            y_sb = youts.tile([O_DIM, BL], BF16)
            for n in range(NCH):
                emit_pending_y()
                sigA_ps = sigpA.tile([128, 4, NC], BF16, tag="sigA_ps")
                sigB_ps = sigpB.tile([128, 2, NC], BF16, tag="sigB_ps")
                g_ps = tgp.tile([128, 2, NC], BF16, tag="g_ps")
                n_acc = KT + (1 if t == 0 else 0) + (1 if use_bias else 0)
                for m in range(MT):
                    if m < 4:
                        dst = sigA_ps[:, m, :]
                    elif m < 6:
                        dst = sigB_ps[:, m - 4, :]
                    else:
                        dst = g_ps[:, m - 6, :]
                    acc = 0
                    for k in range(KT):
                        acc += 1
                        nc.tensor.matmul(
                            dst,
                            wg_sb[:, k, ts(m, 128)],
                            h_in[:, k, ts(n, NC)],
                            start=(acc == 1),
                            stop=(acc == n_acc),
                        )
                    if t == 0:
                        acc += 1
                        nc.tensor.matmul(
                            dst,
                            wx_sb[:, ts(m, 128)],
                            x0_sb[:, ts(n, NC)],
                            start=False,
                            stop=(acc == n_acc),
                        )
                    if use_bias:
                        acc += 1
                        bsrc = bg0_sb if t == 0 else bg_sb
                        nc.tensor.matmul(
                            dst,
                            bsrc[:, ts(m, 128)],
                            ones_sb[:],
                            start=False,
                            stop=(acc == n_acc),
                        )
                sigA_sb = sigs.tile([128, 4, NC], BF16, tag="sigA_sb")
                sigB_sb = sigs.tile([128, 2, NC], BF16, tag="sigB_sb")
                tg_sb = sigs.tile([128, 2, NC], BF16, tag="tg_sb")
                fc_t = tmps.tile([128, KT, NC], F32, tag="fc_t")
                ig_t = tmps.tile([128, KT, NC], BF16, tag="ig_t")

                nc.scalar.activation(sigA_sb[:], sigA_ps[:], AF.Sigmoid)
                nc.scalar.activation(sigB_sb[:], sigB_ps[:], AF.Sigmoid)
                nc.scalar.activation(tg_sb[:], g_ps[:], AF.Tanh)
                nc.vector.tensor_mul(fc_t[:], sigA_sb[:, 2:4, :], cS[:, :, ts(n, NC)])
                nc.vector.tensor_mul(ig_t[:], sigA_sb[:, 0:2, :], tg_sb[:])
                nc.vector.tensor_add(cS[:, :, ts(n, NC)], fc_t[:], ig_t[:])
                # tanh(c') + h are emitted one chunk late so the tanh never
                # waits on this chunk's ADD
                pending_tails.append((t, n, sigB_sb, h_out, y_sb))
                if len(pending_tails) >= 2:
                    emit_tail_one()
        while pending_tails:
            emit_tail_one()
        while pending_ys:
            emit_pending_y()

    return nc


_CACHE: dict = {}


def _get_nc(T: int, use_bias: bool):
    key = (T, use_bias)
    if key not in _CACHE:
        nc = _build(T, use_bias)
        _split_multi_waits(nc)
        _CACHE[key] = nc
    return _CACHE[key]


def _install_trace_shim():
    """Optional: make run_bass_kernel_spmd(trace=True) work in this image
    (missing antenv.axon_hooks). Only used when BASS_LSTM_TRACE=1."""
    so_path = "/opt/axon/libaxon_pjrt.so"
    lib = ctypes.CDLL(so_path)
    if not hasattr(lib, "axon_start_nrt_profile"):
        return
    lib.axon_start_nrt_profile.argtypes = [
        ctypes.POINTER(ctypes.c_int64),
        ctypes.c_size_t,
    ]
    lib.axon_start_nrt_profile.restype = ctypes.c_int64
    lib.axon_stop_nrt_profile.argtypes = [ctypes.c_char_p]
    lib.axon_stop_nrt_profile.restype = ctypes.c_int64

    @contextlib.contextmanager
    def _hook(output_dir, device_ids):
        import jax

        jax.devices()
        if device_ids:
            ids = (ctypes.c_int64 * len(device_ids))(*device_ids)
            rc = lib.axon_start_nrt_profile(ids, len(device_ids))
        else:
            rc = lib.axon_start_nrt_profile(None, 0)
        if rc != 0:
            raise RuntimeError(f"axon_start_nrt_profile rc={rc}")
        try:
            yield
        finally:
            n = lib.axon_stop_nrt_profile(str(output_dir).encode())
            print(f"profile: {n} file(s) written to {output_dir}")

    mod = types.ModuleType("antenv.axon_hooks")
    mod.get_axon_ntff_profile_hook = lambda: _hook
    mod.set_axon_ntff_profile_hook = lambda h: None
    sys.modules["antenv.axon_hooks"] = mod
    import concourse.bass_utils as bu

    bu.upload_artifacts = lambda tmpdir: f"local:{tmpdir}"


LAST_EXEC_TIME_NS = None


def _kernel_impl(h, c, decoder_input, W_ih, W_hh, b_ih, b_hh, fc_W, fc_b, out_len):
    global LAST_EXEC_TIME_NS
    from concourse.bass_utils import run_bass_kernel_spmd

    h = np.asarray(h, np.float32)
    c = np.asarray(c, np.float32)
    decoder_input = np.asarray(decoder_input, np.float32)
    W_ih = np.asarray(W_ih, np.float32)
    W_hh = np.asarray(W_hh, np.float32)
    b_ih = np.asarray(b_ih, np.float32)
    b_hh = np.asarray(b_hh, np.float32)
    fc_W = np.asarray(fc_W, np.float32)
    fc_b = np.asarray(fc_b, np.float32)
    T = int(out_len)

    # fold the y->x feedback into the recurrent weights (exact algebra)
    Wc = (
        W_hh.astype(np.float64) + W_ih.astype(np.float64) @ fc_W.astype(np.float64)
    ).astype(np.float32)
    b_eff = (
        b_ih.astype(np.float64) + b_hh.astype(np.float64)
        + W_ih.astype(np.float64) @ fc_b.astype(np.float64)
    ).astype(np.float32)
    b_0 = (b_ih + b_hh).astype(np.float32)

    # gate rows reordered [i, f, o, g]: sigmoid block first, tanh block last
    perm = np.r_[0:H, H : 2 * H, 3 * H : 4 * H, 2 * H : 3 * H]
    Wc_ord = Wc[perm]
    Wih_ord = W_ih[perm]
    b_ord = b_eff[perm]
    b0_ord = b_0[perm]
    use_bias = bool(np.any(b_ord != 0) or np.any(b0_ord != 0))

    wg = np.ascontiguousarray(Wc_ord.T.reshape(KT, 128, 4 * H)).astype(BF16_NP)
    wy = np.ascontiguousarray(fc_W.T.reshape(KT, 128, O_DIM)).astype(BF16_NP)
    wx = np.ascontiguousarray(Wih_ord.T).astype(BF16_NP)
    h0T = np.ascontiguousarray(h[0].T.reshape(KT, 128, B)).astype(BF16_NP)
    c0T = np.ascontiguousarray(c[0].T.reshape(KT, 128, B), np.float32)
    # The kernel applies Wc (= W_hh + W_ih@fc_W) at every step including t=0,
    # so pre-subtract the spurious t=0 feedback term: with
    # x0' = x0 - h0@fc_W.T, x0'@W_ih.T + h0@Wc.T == x0@W_ih.T + h0@W_hh.T.
    x0_adj = (
        decoder_input[:, 0, :].astype(np.float64)
        - h[0].astype(np.float64) @ fc_W.astype(np.float64).T
    ).astype(np.float32)
    x0T = np.ascontiguousarray(x0_adj.T).astype(BF16_NP)

    in_maps = []
    for ci in range(NCORES):
        cols = slice(ci * BL, (ci + 1) * BL)
        m = {
            "wg": wg,
            "wy": wy,
            "wx": wx,
            "h0": np.ascontiguousarray(h0T[:, :, cols]),
            "c0": np.ascontiguousarray(c0T[:, :, cols]),
            "x0": np.ascontiguousarray(x0T[:, cols]),
        }
        if use_bias:
            m["bg"] = np.ascontiguousarray(b_ord.reshape(1, 4 * H))
            m["bg0"] = np.ascontiguousarray(b0_ord.reshape(1, 4 * H))
        in_maps.append(m)

    nc = _get_nc(T, use_bias)
    trace = os.environ.get("BASS_LSTM_TRACE") == "1"
    if trace:
        try:
            _install_trace_shim()
        except Exception as e:  # profiling is best-effort
            print("trace shim failed:", e)
            trace = False
    res = None
    last_exc = None
    for attempt in range(2):
        try:
            res = run_bass_kernel_spmd(
                nc, in_maps, core_ids=list(range(NCORES)), trace=trace
            )
            break
        except Exception as e:
            # transient NRT_EXEC_UNIT_UNRECOVERABLE is sometimes seen on the
            # first execution after a fresh compile; the PJRT client may be
            # poisoned afterwards, so tear it down and retry once in-process
            # (the caller falls back to fresh subprocesses below).
            last_exc = e
            trace = False
            try:
                import jax

                jax.clear_caches()
                jax.extend.backend.clear_backends()
            except Exception:
                pass
            import time as _time

            _time.sleep(10)
    if res is None:
        raise last_exc
    LAST_EXEC_TIME_NS = res.exec_time_ns

    out = np.empty((B, T, O_DIM), np.float32)
    for ci in range(NCORES):
        ys = np.asarray(res.results[ci]["ys"], dtype=np.float32)  # [T, O, BL]
        out[ci * BL : (ci + 1) * BL] = np.transpose(ys, (2, 0, 1))
    out += fc_b  # fc bias applied on host (exact, broadcast over last dim)
    return out


def kernel(h, c, decoder_input, W_ih, W_hh, b_ih, b_hh, fc_W, fc_b, out_len):
    """Entry point. Runs on this process's jax/axon client; if the device
    crashes (rare transient after a fresh compile poisons the PJRT client),
    re-runs in fresh subprocesses with backoff — those hit the NEFF cache
    populated by this process's compile and have been reliable."""
    global LAST_EXEC_TIME_NS
    args = dict(
        h=h, c=c, decoder_input=decoder_input, W_ih=W_ih, W_hh=W_hh,
        b_ih=b_ih, b_hh=b_hh, fc_W=fc_W, fc_b=fc_b, out_len=out_len,
    )
    try:
        return _kernel_impl(**args)
    except Exception:
        pass

    import subprocess
    import tempfile
    import time

    my_path = os.path.abspath(__file__)
    for attempt, delay in enumerate((20.0, 60.0, 120.0, 180.0)):
        time.sleep(delay)
        tmpdir = tempfile.mkdtemp(prefix="lstmkrn_")
        in_file = os.path.join(tmpdir, "in.npz")
        out_file = os.path.join(tmpdir, "out.npz")
        np.savez(
            in_file,
            **{k: np.asarray(v) for k, v in args.items()},
        )
        r = subprocess.run(
            [sys.executable, my_path, "--child", in_file, out_file],
            capture_output=True,
            timeout=3600,
        )
        if r.returncode == 0 and os.path.exists(out_file):
            d = np.load(out_file)
            t = float(d["exec_time_ns"])
            LAST_EXEC_TIME_NS = int(t) if t >= 0 else None
            return d["out"]
    raise RuntimeError(
        "kernel execution failed in-process and in all subprocess retries"
    )


def _child_main(in_file, out_file):
    d = np.load(in_file)
    args = {k: d[k] for k in d.files}
    args["out_len"] = int(args["out_len"])
    out = _kernel_impl(**args)
    t = LAST_EXEC_TIME_NS if LAST_EXEC_TIME_NS is not None else -1
    np.savez(out_file, out=out, exec_time_ns=np.int64(t))


if __name__ == "__main__":
    if len(sys.argv) == 4 and sys.argv[1] == "--child":
        _child_main(sys.argv[2], sys.argv[3])


# revision 32
# speedup vs baseline: 1.0187x; 1.0187x over previous
"""LSTM decoder (B=16384, H=256, I=O=2, T=60) on 8 TRN2 NeuronCores.

Strategy: pure data parallel over the batch dim (2048 rows/core). All state
kept on-chip in transposed [H, B_local] layout so the recurrent matmul needs
no transposes. The output-feedback path (y_t -> x_{t+1}) is folded into the
recurrent weights: for t>=1,
    gates_t = h_t @ (W_hh + W_ih @ fc_W).T + (b_ih + b_hh + W_ih @ fc_b)
so each step is one [2048,256]@[256,1024] matmul (+ tiny fc head for y_t).
Gate rows are reordered to [i, f, o, g]; PSUM is split 4+2+2 banks
(i,f | o | g + y-head) so TensorE refills one pool while ScalarE drains
another, and the tanh(c)/h/y tail is software-pipelined 1-2 chunks behind
the gate matmuls. ScalarE (5 transcendentals per hidden element) is the
bottleneck engine at ~96% busy; measured ~1.21 ms for the full decode.
"""

import os
import sys
import types
import contextlib
import ctypes

sys.path.insert(0, "/opt/trn_rl_repo")

import numpy as np
import ml_dtypes

BF16_NP = ml_dtypes.bfloat16

import concourse.bass as bass
import concourse.tile as tile
from concourse import mybir
from concourse.bass import ts
from concourse.vector_clock import ScopedClock
import bass_rust

F32 = mybir.dt.float32
BF16 = mybir.dt.bfloat16
AF = mybir.ActivationFunctionType

B, H, I_DIM, O_DIM = 16384, 256, 2, 2
NCORES = 8
BL = B // NCORES          # 2048 batch rows per core
NC = 1024                 # batch chunk (one PSUM bank of bf16 gate outputs)
NCH = BL // NC            # 2 chunks per step
KT = H // 128             # 2 contraction tiles
MT = 4 * H // 128         # 8 gate row tiles

_MAX_WAITS = 1  # walrus in this container rejects >1 sem wait per instruction

TileContextFixed = tile.TileContext


def _split_multi_waits(nc, max_waits=_MAX_WAITS):
    """The pinned walrus rejects instructions carrying more than one
    semaphore wait. Split extras onto same-engine NoOps placed just before
    the instruction (same stream order => same semantics)."""
    ctr = 0
    for fn in nc.m.functions:
        for bb in fn.blocks:
            out = []
            changed = False
            for inst in bb.instructions:
                si = getattr(inst, "sync_info", None)
                waits = list(si.on_wait) if si is not None and si.on_wait else []
                if (
                    len(waits) > max_waits
                    and inst.engine != mybir.EngineType.Unassigned
                ):
                    keep = waits[-max_waits:]
                    extra = waits[:-max_waits]
                    for j in range(0, len(extra), max_waits):
                        ctr += 1
                        nop = mybir.InstNoOp(
                            name=f"waitsplit_{ctr}", engine=inst.engine
                        )
                        nop.sync_info = bass_rust.SyncInfo(
                            on_wait=extra[j : j + max_waits], on_update=[]
                        )
                        try:
                            nc.register_instruction(nop)
                        except Exception:
                            pass
                        out.append(nop)
                    si.on_wait = keep
                    inst.sync_info = si
                    changed = True
                out.append(inst)
            if changed:
                bb.instructions = out


def _build(T: int, use_bias: bool):
    nc = bass.Bass()

    wg_d = nc.declare_dram_parameter("wg", [KT, 128, 4 * H], BF16, isOutput=False)
    wy_d = nc.declare_dram_parameter("wy", [KT, 128, O_DIM], BF16, isOutput=False)
    wx_d = nc.declare_dram_parameter("wx", [I_DIM, 4 * H], BF16, isOutput=False)
    h0_d = nc.declare_dram_parameter("h0", [KT, 128, BL], BF16, isOutput=False)
    c0_d = nc.declare_dram_parameter("c0", [KT, 128, BL], F32, isOutput=False)
    x0_d = nc.declare_dram_parameter("x0", [I_DIM, BL], BF16, isOutput=False)
    if use_bias:
        bg_d = nc.declare_dram_parameter("bg", [1, 4 * H], F32, isOutput=False)
        bg0_d = nc.declare_dram_parameter("bg0", [1, 4 * H], F32, isOutput=False)
    ys_d = nc.declare_dram_parameter("ys", [T, O_DIM, BL], BF16, isOutput=True)

    with contextlib.ExitStack() as ctx:
        tc = ctx.enter_context(TileContextFixed(nc))
        consts = ctx.enter_context(tc.tile_pool(name="consts", bufs=1))
        state = ctx.enter_context(tc.tile_pool(name="state", bufs=1))
        # PSUM: 4 banks (i,f) + 2 banks (o) + 2 banks (g, shared with y head)
        sigpA = ctx.enter_context(tc.tile_pool(name="sigpA", bufs=1, space="PSUM"))
        sigpB = ctx.enter_context(tc.tile_pool(name="sigpB", bufs=1, space="PSUM"))
        tgp = ctx.enter_context(tc.tile_pool(name="tgp", bufs=1, space="PSUM"))
        sigs = ctx.enter_context(tc.tile_pool(name="sigs", bufs=4))
        tmps = ctx.enter_context(tc.tile_pool(name="tmps", bufs=3))
        youts = ctx.enter_context(tc.tile_pool(name="youts", bufs=2))

        # PE warmup: ~5us of junk matmuls with no DMA dependency, so the HAM
        # clock-gate reaches 2.4 GHz before the first real gate matmul.
        warm_sb = consts.tile([128, NC], BF16)
        nc.vector.memset(warm_sb[:], 0.0)
        warm_ps = tgp.tile([128, NC], BF16, tag="g_ps")
        for _ in range(16):
            nc.tensor.matmul(
                warm_ps[:], warm_sb[:, 0:128], warm_sb[:], start=True, stop=True
            )

        wg_sb = consts.tile([128, KT, 4 * H], BF16)
        wy_sb = consts.tile([128, KT, O_DIM], BF16)
        wx_sb = consts.tile([I_DIM, 4 * H], BF16)
        x0_sb = consts.tile([I_DIM, BL], BF16)
        for k in range(KT):
            nc.sync.dma_start(out=wg_sb[:, k, :], in_=wg_d[k])
        nc.sync.dma_start(out=wx_sb[:], in_=wx_d[:])
        nc.sync.dma_start(out=x0_sb[:], in_=x0_d[:])
        for k in range(KT):
            nc.sync.dma_start(out=wy_sb[:, k, :], in_=wy_d[k])
        if use_bias:
            bg_sb = consts.tile([1, 4 * H], F32)
            bg0_sb = consts.tile([1, 4 * H], F32)
            ones_sb = consts.tile([1, NC], F32)
            nc.sync.dma_start(out=bg_sb[:], in_=bg_d[:])
            nc.sync.dma_start(out=bg0_sb[:], in_=bg0_d[:])
            nc.vector.memset(ones_sb[:], 1.0)

        hA = state.tile([128, KT, BL], BF16, tag="hA")
        hB = state.tile([128, KT, BL], BF16, tag="hB")
        cS = state.tile([128, KT, BL], F32, tag="cS")
        # chunked, h/c-interleaved loads so step-0 compute (and its c-path)
        # starts as soon as the first slices land
        for n in range(NCH):
            for k in range(KT):
                nc.sync.dma_start(
                    out=hA[:, k, ts(n, NC)], in_=h0_d[k, :, ts(n, NC)]
                )
            for k in range(KT):
                nc.sync.dma_start(
                    out=cS[:, k, ts(n, NC)], in_=c0_d[k, :, ts(n, NC)]
                )

        # Software pipeline: tanh(c)/h of chunks (t, 2j..2j+1) are emitted as a
        # pair after both adds, and the y head trails one further chunk, so
        # ScalarE/VectorE never stall the gate pipeline.
        pending_tails = []  # [(t, n, o_sb, h_out, y_sb), ...]
        pending_ys = []     # fifo of (h_buf, t, n, y_sb)

        def emit_pending_y():
            if not pending_ys:
                return
            h_src, t_p, n_p, y_sb_p = pending_ys.pop(0)
            y_ps = tgp.tile([O_DIM, NC], BF16, tag="g_ps")
            for k in range(KT):
                nc.tensor.matmul(
                    y_ps[:],
                    wy_sb[:, k, :],
                    h_src[:, k, ts(n_p, NC)],
                    start=(k == 0),
                    stop=(k == KT - 1),
                )
            nc.vector.tensor_copy(y_sb_p[:, ts(n_p, NC)], y_ps[:])
            if n_p == NCH - 1:
                nc.sync.dma_start(out=ys_d[t_p], in_=y_sb_p[:])

        def emit_tail_one():
            # tanh(c') + h for the oldest pending chunk (runs one chunk late)
            t_p, n_p, o_sb_p, h_out_p, y_sb_p = pending_tails.pop(0)
            tc_t = tmps.tile([128, KT, NC], BF16, tag="tc_t")
            nc.scalar.activation(tc_t[:], cS[:, :, ts(n_p, NC)], AF.Tanh)
            nc.vector.tensor_mul(
                h_out_p[:, :, ts(n_p, NC)], o_sb_p[:], tc_t[:]
            )
            pending_ys.append((h_out_p, t_p, n_p, y_sb_p))

        y_sb = None
        for t in range(T):
            h_in = hA if t % 2 == 0 else hB
            h_out = hB if t % 2 == 0 else hA
            y_sb = youts.tile([O_DIM, BL], BF16)
            for n in range(NCH):
                emit_pending_y()
                sigA_ps = sigpA.tile([128, 4, NC], BF16, tag="sigA_ps")
                sigB_ps = sigpB.tile([128, 2, NC], BF16, tag="sigB_ps")
                g_ps = tgp.tile([128, 2, NC], BF16, tag="g_ps")
                n_acc = KT + (1 if t == 0 else 0) + (1 if use_bias else 0)
                for m in range(MT):
                    if m < 4:
                        dst = sigA_ps[:, m, :]
                    elif m < 6:
                        dst = sigB_ps[:, m - 4, :]
                    else:
                        dst = g_ps[:, m - 6, :]
                    acc = 0
                    for k in range(KT):
                        acc += 1
                        nc.tensor.matmul(
                            dst,
                            wg_sb[:, k, ts(m, 128)],
                            h_in[:, k, ts(n, NC)],
                            start=(acc == 1),
                            stop=(acc == n_acc),
                        )
                    if t == 0:
                        acc += 1
                        nc.tensor.matmul(
                            dst,
                            wx_sb[:, ts(m, 128)],
                            x0_sb[:, ts(n, NC)],
                            start=False,
                            stop=(acc == n_acc),
                        )
                    if use_bias:
                        acc += 1
                        bsrc = bg0_sb if t == 0 else bg_sb
                        nc.tensor.matmul(
                            dst,
                            bsrc[:, ts(m, 128)],
                            ones_sb[:],
                            start=False,
                            stop=(acc == n_acc),
                        )
                sigA_sb = sigs.tile([128, 4, NC], BF16, tag="sigA_sb")
                sigB_sb = sigs.tile([128, 2, NC], BF16, tag="sigB_sb")
                tg_sb = sigs.tile([128, 2, NC], BF16, tag="tg_sb")
                fc_t = tmps.tile([128, KT, NC], F32, tag="fc_t")
                ig_t = tmps.tile([128, KT, NC], BF16, tag="ig_t")

                nc.scalar.activation(sigA_sb[:], sigA_ps[:], AF.Sigmoid)
                nc.scalar.activation(sigB_sb[:], sigB_ps[:], AF.Sigmoid)
                nc.scalar.activation(tg_sb[:], g_ps[:], AF.Tanh)
                nc.vector.tensor_mul(fc_t[:], sigA_sb[:, 2:4, :], cS[:, :, ts(n, NC)])
                nc.vector.tensor_mul(ig_t[:], sigA_sb[:, 0:2, :], tg_sb[:])
                nc.vector.tensor_add(cS[:, :, ts(n, NC)], fc_t[:], ig_t[:])
                # tanh(c') + h are emitted one chunk late so the tanh never
                # waits on this chunk's ADD
                pending_tails.append((t, n, sigB_sb, h_out, y_sb))
                if len(pending_tails) >= 2:
                    emit_tail_one()
        while pending_tails:
            emit_tail_one()
        while pending_ys:
            emit_pending_y()

    return nc


_CACHE: dict = {}


def _get_nc(T: int, use_bias: bool):
    key = (T, use_bias)
    if key not in _CACHE:
        nc = _build(T, use_bias)
        _split_multi_waits(nc)
        _CACHE[key] = nc
    return _CACHE[key]


def _install_trace_shim():
    """Optional: make run_bass_kernel_spmd(trace=True) work in this image
    (missing antenv.axon_hooks). Only used when BASS_LSTM_TRACE=1."""
    so_path = "/opt/axon/libaxon_pjrt.so"
    lib = ctypes.CDLL(so_path)
    if not hasattr(lib, "axon_start_nrt_profile"):
        return
    lib.axon_start_nrt_profile.argtypes = [
        ctypes.POINTER(ctypes.c_int64),
        ctypes.c_size_t,
    ]
    lib.axon_start_nrt_profile.restype = ctypes.c_int64
    lib.axon_stop_nrt_profile.argtypes = [ctypes.c_char_p]
    lib.axon_stop_nrt_profile.restype = ctypes.c_int64

    @contextlib.contextmanager
    def _hook(output_dir, device_ids):
        import jax

        jax.devices()
        if device_ids:
            ids = (ctypes.c_int64 * len(device_ids))(*device_ids)
            rc = lib.axon_start_nrt_profile(ids, len(device_ids))
        else:
            rc = lib.axon_start_nrt_profile(None, 0)
        if rc != 0:
            raise RuntimeError(f"axon_start_nrt_profile rc={rc}")
        try:
            yield
        finally:
            n = lib.axon_stop_nrt_profile(str(output_dir).encode())
            print(f"profile: {n} file(s) written to {output_dir}")

    mod = types.ModuleType("antenv.axon_hooks")
    mod.get_axon_ntff_profile_hook = lambda: _hook
    mod.set_axon_ntff_profile_hook = lambda h: None
    sys.modules["antenv.axon_hooks"] = mod
    import concourse.bass_utils as bu

    bu.upload_artifacts = lambda tmpdir: f"local:{tmpdir}"


LAST_EXEC_TIME_NS = None


def _kernel_impl(h, c, decoder_input, W_ih, W_hh, b_ih, b_hh, fc_W, fc_b, out_len):
    global LAST_EXEC_TIME_NS
    from concourse.bass_utils import run_bass_kernel_spmd

    h = np.asarray(h, np.float32)
    c = np.asarray(c, np.float32)
    decoder_input = np.asarray(decoder_input, np.float32)
    W_ih = np.asarray(W_ih, np.float32)
    W_hh = np.asarray(W_hh, np.float32)
    b_ih = np.asarray(b_ih, np.float32)
    b_hh = np.asarray(b_hh, np.float32)
    fc_W = np.asarray(fc_W, np.float32)
    fc_b = np.asarray(fc_b, np.float32)
    T = int(out_len)

    # fold the y->x feedback into the recurrent weights (exact algebra)
    Wc = (
        W_hh.astype(np.float64) + W_ih.astype(np.float64) @ fc_W.astype(np.float64)
    ).astype(np.float32)
    b_eff = (
        b_ih.astype(np.float64) + b_hh.astype(np.float64)
        + W_ih.astype(np.float64) @ fc_b.astype(np.float64)
    ).astype(np.float32)
    b_0 = (b_ih + b_hh).astype(np.float32)

    # gate rows reordered [i, f, o, g]: sigmoid block first, tanh block last
    perm = np.r_[0:H, H : 2 * H, 3 * H : 4 * H, 2 * H : 3 * H]
    Wc_ord = Wc[perm]
    Wih_ord = W_ih[perm]
    b_ord = b_eff[perm]
    b0_ord = b_0[perm]
    use_bias = bool(np.any(b_ord != 0) or np.any(b0_ord != 0))

    wg = np.ascontiguousarray(Wc_ord.T.reshape(KT, 128, 4 * H)).astype(BF16_NP)
    wy = np.ascontiguousarray(fc_W.T.reshape(KT, 128, O_DIM)).astype(BF16_NP)
    wx = np.ascontiguousarray(Wih_ord.T).astype(BF16_NP)
    h0T = np.ascontiguousarray(h[0].T.reshape(KT, 128, B)).astype(BF16_NP)
    c0T = np.ascontiguousarray(c[0].T.reshape(KT, 128, B), np.float32)
    # The kernel applies Wc (= W_hh + W_ih@fc_W) at every step including t=0,
    # so pre-subtract the spurious t=0 feedback term: with
    # x0' = x0 - h0@fc_W.T, x0'@W_ih.T + h0@Wc.T == x0@W_ih.T + h0@W_hh.T.
    x0_adj = (
        decoder_input[:, 0, :].astype(np.float64)
        - h[0].astype(np.float64) @ fc_W.astype(np.float64).T
    ).astype(np.float32)
    x0T = np.ascontiguousarray(x0_adj.T).astype(BF16_NP)

    in_maps = []
    for ci in range(NCORES):
        cols = slice(ci * BL, (ci + 1) * BL)
        m = {
            "wg": wg,
            "wy": wy,
            "wx": wx,
            "h0": np.ascontiguousarray(h0T[:, :, cols]),
            "c0": np.ascontiguousarray(c0T[:, :, cols]),
            "x0": np.ascontiguousarray(x0T[:, cols]),
        }
        if use_bias:
            m["bg"] = np.ascontiguousarray(b_ord.reshape(1, 4 * H))
            m["bg0"] = np.ascontiguousarray(b0_ord.reshape(1, 4 * H))
        in_maps.append(m)

    nc = _get_nc(T, use_bias)
    trace = os.environ.get("BASS_LSTM_TRACE") == "1"
    if trace:
        try:
            _install_trace_shim()
        except Exception as e:  # profiling is best-effort
            print("trace shim failed:", e)
            trace = False
    res = None
    last_exc = None
    for attempt in range(2):
        try:
            res = run_bass_kernel_spmd(
                nc, in_maps, core_ids=list(range(NCORES)), trace=trace
            )
            break
        except Exception as e:
            # transient NRT_EXEC_UNIT_UNRECOVERABLE is sometimes seen on the
            # first execution after a fresh compile; the PJRT client may be
            # poisoned afterwards, so tear it down and retry once in-process
            # (the caller falls back to fresh subprocesses below).
            last_exc = e
            trace = False
            try:
                import jax

                jax.clear_caches()
                jax.extend.backend.clear_backends()
            except Exception:
                pass
            import time as _time

            _time.sleep(10)
    if res is None:
        raise last_exc
    LAST_EXEC_TIME_NS = res.exec_time_ns

    out = np.empty((B, T, O_DIM), np.float32)
    for ci in range(NCORES):
        ys = np.asarray(res.results[ci]["ys"], dtype=np.float32)  # [T, O, BL]
        out[ci * BL : (ci + 1) * BL] = np.transpose(ys, (2, 0, 1))
    out += fc_b  # fc bias applied on host (exact, broadcast over last dim)
    return out


def kernel(h, c, decoder_input, W_ih, W_hh, b_ih, b_hh, fc_W, fc_b, out_len):
    """Entry point. Runs on this process's jax/axon client; if the device
    crashes (rare transient after a fresh compile poisons the PJRT client),
    re-runs in fresh subprocesses with backoff — those hit the NEFF cache
    populated by this process's compile and have been reliable."""
    global LAST_EXEC_TIME_NS
    args = dict(
        h=h, c=c, decoder_input=decoder_input, W_ih=W_ih, W_hh=W_hh,
        b_ih=b_ih, b_hh=b_hh, fc_W=fc_W, fc_b=fc_b, out_len=out_len,
    )
    try:
        return _kernel_impl(**args)
    except Exception:
        pass

    import subprocess
    import tempfile
    import time

    my_path = os.path.abspath(__file__)
    for attempt, delay in enumerate((20.0, 60.0, 120.0, 180.0)):
        time.sleep(delay)
        tmpdir = tempfile.mkdtemp(prefix="lstmkrn_")
        in_file = os.path.join(tmpdir, "in.npz")
        out_file = os.path.join(tmpdir, "out.npz")
        np.savez(
            in_file,
            **{k: np.asarray(v) for k, v in args.items()},
        )
        r = subprocess.run(
            [sys.executable, my_path, "--child", in_file, out_file],
            capture_output=True,
            timeout=3600,
        )
        if r.returncode == 0 and os.path.exists(out_file):
            d = np.load(out_file)
            t = float(d["exec_time_ns"])
            LAST_EXEC_TIME_NS = int(t) if t >= 0 else None
            return d["out"]
    raise RuntimeError(
        "kernel execution failed in-process and in all subprocess retries"
    )


def _child_main(in_file, out_file):
    d = np.load(in_file)
    args = {k: d[k] for k in d.files}
    args["out_len"] = int(args["out_len"])
    out = _kernel_impl(**args)
    t = LAST_EXEC_TIME_NS if LAST_EXEC_TIME_NS is not None else -1
    np.savez(out_file, out=out, exec_time_ns=np.int64(t))


if __name__ == "__main__":
    if len(sys.argv) == 4 and sys.argv[1] == "--child":
        _child_main(sys.argv[2], sys.argv[3])
